# revision 1
# baseline (speedup 1.0000x reference)
"""ChebConv(K=3) x3 GNN encoder on 8 trn2 NeuronCores (Bass/Tile).

Self-contained: inlines the persistent PJRT runner + program builders.
Strategy: shard nodes/edges by destination across 8 cores; per-core
98 blocks x 128 dst slots; per 128-edge tile an indirect-DMA row gather
plus a one-hot selection matrix (iota==dloc)*w built on DVE, reduced on
the PE via P^T @ M with PSUM accumulation; dense matmuls for the
Chebyshev combine; host all-gathers activations between stages.
"""
import numpy as np
import jax
from jax.sharding import Mesh, PartitionSpec, NamedSharding
from jax.experimental.shard_map import shard_map

import concourse.bass as bass
import concourse.bacc as bacc
import concourse.mybir as mybir
from concourse.tile import TileContext
from concourse import bass2jax


import numpy as np
import jax
from jax.sharding import Mesh, PartitionSpec, NamedSharding
from jax.experimental.shard_map import shard_map

import concourse.mybir as mybir
from concourse import bass2jax


class Runner:
    def __init__(self, nc, n_cores=8):
        bass2jax.install_neuronx_cc_hook()
        self.nc = nc
        self.n_cores = n_cores
        partition_name = (
            nc.partition_id_tensor.name if nc.partition_id_tensor else None
        )
        in_names, out_names, out_avals = [], [], []
        for alloc in nc.m.functions[0].allocations:
            if not isinstance(alloc, mybir.MemoryLocationSet):
                continue
            name = alloc.memorylocations[0].name
            if alloc.kind == "ExternalInput":
                if name != partition_name:
                    in_names.append(name)
            elif alloc.kind == "ExternalOutput":
                out_names.append(name)
                out_avals.append(
                    jax.core.ShapedArray(
                        tuple(alloc.tensor_shape), mybir.dt.np(alloc.dtype)
                    )
                )
        self.in_names, self.out_names, self.out_avals = in_names, out_names, out_avals
        n_params = len(in_names)
        all_in_names = in_names + out_names + (
            [partition_name] if partition_name else []
        )

        def _body(*args):
            operands = list(args)
            if partition_name is not None:
                operands.append(bass2jax.partition_id_tensor())
            outs = bass2jax._bass_exec_p.bind(
                *operands,
                out_avals=tuple(out_avals),
                in_names=tuple(all_in_names),
                out_names=tuple(out_names),
                lowering_input_output_aliases=(),
                sim_require_finite=True,
                sim_require_nnan=True,
                nc=nc,
            )
            return tuple(outs)

        devices = jax.devices()[:n_cores]
        self.mesh = Mesh(np.asarray(devices), ("core",))
        self.sharding = NamedSharding(self.mesh, PartitionSpec("core"))
        nin = n_params + len(out_names)
        self.fn = jax.jit(
            shard_map(
                _body,
                mesh=self.mesh,
                in_specs=(PartitionSpec("core"),) * nin,
                out_specs=(PartitionSpec("core"),) * len(out_names),
                check_rep=False,
            ),
            keep_unused=True,
        )

    def put_inputs(self, in_maps):
        """in_maps: list of dicts (len n_cores) or single dict (replicated).
        Returns device-resident concatenated input list."""
        if isinstance(in_maps, dict):
            in_maps = [in_maps] * self.n_cores
        concat = [
            np.concatenate([np.asarray(m[n]) for m in in_maps], axis=0)
            for n in self.in_names
        ]
        return [jax.device_put(a, self.sharding) for a in concat]

    def zeros(self):
        return [
            jax.device_put(
                np.zeros((self.n_cores * a.shape[0], *a.shape[1:]), a.dtype),
                self.sharding,
            )
            for a in self.out_avals
        ]

    def __call__(self, dev_in, dev_zero=None):
        if dev_zero is None:
            dev_zero = self.zeros()
        outs = self.fn(*dev_in, *dev_zero)
        jax.block_until_ready(outs)
        return outs

    def results(self, outs):
        """Split outputs back to per-core dicts."""
        res = []
        for c in range(self.n_cores):
            d = {}
            for i, name in enumerate(self.out_names):
                a = np.asarray(outs[i])
                d[name] = a.reshape(self.n_cores, *self.out_avals[i].shape)[c]
            res.append(d)
        return res



import concourse.bass as bass
import concourse.bacc as bacc
import concourse.mybir as mybir
from concourse.tile import TileContext

F32 = mybir.dt.float32
I32 = mybir.dt.int32
NCORES = 8


class Cfg:
    def __init__(self, n_nodes, npc, blk=128):
        assert npc * NCORES == n_nodes
        self.N = n_nodes
        self.NPC = npc
        self.BLK = blk
        self.NB = -(-npc // blk)          # blocks per core
        self.SLOTS = self.NB * blk        # slots per core (>= npc)


# ---------------------------------------------------------------- host prep

def host_prep(cfg, edge_index, backend="indirect", n_chunks=1, remap_slots=False):
    """Bin edges by (core, block), pad to T_fix tiles, build packed meta.

    backend="indirect": returns (metas, T_fix). metas[c] is
      [NB*128, 3*T_fix] f32; cols [0:T) src ids (int32 bitcast),
      [T:2T) dloc f32, [2T:3T) w f32.

    backend="dma_gather": edges additionally grouped by src chunk
      (n_chunks windows of cfg.N), each (block, chunk) group padded to
      whole 128-edge tiles. Returns (metas, idx16s, T_chunks, T_fix):
      idx16s[c] is [NB*128, T_fix*8] int16 in the wrapped-16 layout
      (cols for chunk ch start at tile offset sum(T_chunks[:ch])*8),
      T_chunks[ch] = tiles per chunk (global max, SPMD-uniform).
    """
    N, NPC, BLK, NB = cfg.N, cfg.NPC, cfg.BLK, cfg.NB
    src = np.asarray(edge_index[0], dtype=np.int64)
    dst = np.asarray(edge_index[1], dtype=np.int64)
    mask = src != dst
    deg = np.bincount(src[mask], minlength=N).astype(np.float32)
    dinv = np.where(deg > 0, (1.0 / np.sqrt(np.maximum(deg, 1.0))).astype(np.float32), 0.0).astype(np.float32)
    w_all = (-dinv[src] * dinv[dst]).astype(np.float32)

    src = src[mask]
    dst = dst[mask]
    w = w_all[mask]

    order = np.argsort(dst, kind="stable")
    src, dst, w = src[order], dst[order], w[order]

    if remap_slots:
        # gather-source rows live in per-core slot layout [NCORES*SLOTS, C]
        src = (src // NPC) * cfg.SLOTS + (src % NPC)

    core = dst // NPC
    core_starts = np.searchsorted(core, np.arange(NCORES + 1))
    blk_of = ((dst - core * NPC) // BLK).astype(np.int64)

    NG = NCORES * cfg.SLOTS if remap_slots else N   # gather-space rows
    CH = -(-NG // n_chunks)
    if backend == "dma_gather":
        assert CH <= 32768, f"chunk {CH} exceeds int16 reach"

    # group edges per (core, block); within block sort by src chunk
    groups = []      # per core: list over blocks of (src, dloc, w) arrays
    cnt_bc = np.zeros((NCORES, NB, n_chunks), np.int64)
    for c in range(NCORES):
        s, e = core_starts[c], core_starts[c + 1]
        cs, cd, cw = src[s:e], dst[s:e], w[s:e]
        b = (cd - c * NPC) // BLK
        dloc = (cd - c * NPC) % BLK
        bstart = np.searchsorted(b, np.arange(NB + 1))
        per_blk = []
        for bi in range(NB):
            s0, e0 = bstart[bi], bstart[bi + 1]
            bs, bd, bw = cs[s0:e0], dloc[s0:e0], cw[s0:e0]
            o2 = np.argsort(bs // CH, kind="stable")
            bs, bd, bw = bs[o2], bd[o2], bw[o2]
            per_blk.append((bs, bd, bw))
            cnt_bc[c, bi] = np.bincount(bs // CH, minlength=n_chunks)
        groups.append(per_blk)

    if backend == "indirect":
        T_fix = int(-(-cnt_bc.sum(axis=2).max() // 128))
        metas = []
        for c in range(NCORES):
            meta = np.zeros((NB, 128, 3 * T_fix), np.float32)
            idx_i32 = np.zeros((NB, 128, T_fix), np.int32)
            for bi in range(NB):
                bs, bd, bw = groups[c][bi]
                n = len(bs)
                if n == 0:
                    continue
                t_i = np.arange(n) // 128
                p_i = np.arange(n) % 128
                idx_i32[bi, p_i, t_i] = bs.astype(np.int32)
                meta[bi, p_i, T_fix + t_i] = bd.astype(np.float32)
                meta[bi, p_i, 2 * T_fix + t_i] = bw
            meta[:, :, 0:T_fix] = idx_i32.view(np.float32)
            metas.append(meta.reshape(NB * 128, 3 * T_fix))
        return metas, T_fix

    assert backend == "dma_gather"
    # per-chunk global max tile count (SPMD-uniform)
    T_chunks = [int(-(-cnt_bc[:, :, ch].max() // 128)) for ch in range(n_chunks)]
    T_chunks = [max(t, 1) for t in T_chunks]
    T_fix = sum(T_chunks)
    toff = np.concatenate([[0], np.cumsum(T_chunks)])
    metas, idx16s = [], []
    for c in range(NCORES):
        meta = np.zeros((NB, 128, 3 * T_fix), np.float32)
        idx16 = np.zeros((NB, n_chunks), object)
        idx_out = np.zeros((NB, 128, T_fix * 8), np.int16)
        for bi in range(NB):
            bs, bd, bw = groups[c][bi]
            chb = np.searchsorted(bs // CH, np.arange(n_chunks + 1))
            for ch in range(n_chunks):
                s0, e0 = chb[ch], chb[ch + 1]
                gs = bs[s0:e0] - ch * CH
                gd, gw = bd[s0:e0], bw[s0:e0]
                npad = T_chunks[ch] * 128
                flat = np.zeros(npad, np.int16)
                flat[:len(gs)] = gs.astype(np.int16)
                # wrapped-16 layout, replicated over 8 partition groups
                wrapped = flat.reshape(npad // 16, 16).T  # [16, npad/16]
                idx_out[bi, :, toff[ch] * 8:(toff[ch] + T_chunks[ch]) * 8] = \
                    np.tile(wrapped, (8, 1))
                # meta (dloc/w) at tile offset toff[ch]
                n = len(gs)
                if n:
                    t_i = np.arange(n) // 128 + toff[ch]
                    p_i = np.arange(n) % 128
                    meta[bi, p_i, T_fix + t_i] = gd.astype(np.float32)
                    meta[bi, p_i, 2 * T_fix + t_i] = gw
        metas.append(meta.reshape(NB * 128, 3 * T_fix))
        idx16s.append(idx_out.reshape(NB * 128, T_fix * 8))
    return metas, idx16s, T_chunks, T_fix


def iota_host():
    return np.broadcast_to(np.arange(128, dtype=np.float32), (128, 128)).copy()


# ------------------------------------------------------------- programs

def _p_build(nc, P_t, iota, m, T, t):
    """P_t[p, c] = (iota[c] == dloc[p]) * w[p]"""
    nc.vector.tensor_scalar(
        out=P_t[:], in0=iota[:],
        scalar1=m[:, T + t:T + t + 1],
        scalar2=m[:, 2 * T + t:2 * T + t + 1],
        op0=mybir.AluOpType.is_equal,
        op1=mybir.AluOpType.mult,
    )


def _emit_gathers(nc, cfg, backend, gather_cfg, pools, v_d, m, i, T, C, ix=None):
    """Emit per-block gathers; returns list of T [128, C] APs (tile slices)."""
    gpool = pools
    if backend == "indirect":
        gs = []
        for t in range(T):
            g = gpool.tile([128, C], F32, tag=f"g{t}")
            nc.gpsimd.indirect_dma_start(
                out=g[:], out_offset=None, in_=v_d[:],
                in_offset=bass.IndirectOffsetOnAxis(
                    ap=m[:, t:t + 1].bitcast(I32), axis=0),
            )
            gs.append(g)
        return [g[:] for g in gs]
    # dma_gather backend: one instruction per src chunk
    T_chunks, CH = gather_cfg
    g = gpool.tile([128, T * C], F32, tag="gall")
    toff = 0
    for ch, T_ch in enumerate(T_chunks):
        lo = ch * CH
        hi = min(lo + CH, v_d.shape[0])
        nc.gpsimd.dma_gather(
            g[:, toff * C:(toff + T_ch) * C].rearrange("p (t c) -> p t c", c=C),
            v_d[lo:hi, :],
            ix[:, toff * 8:(toff + T_ch) * 8],
            T_ch * 128, T_ch * 128, C,
        )
        toff += T_ch
    return [g[:, t * C:(t + 1) * C] for t in range(T)]


def build_prop(cfg, C, T, unroll=2, backend="indirect", gather_cfg=None):
    """Program A: y[slots, C] = prop(v) for this core's dst slots."""
    NB, SLOTS = cfg.NB, cfg.SLOTS
    nc = bacc.Bacc("TRN2", target_bir_lowering=False, debug=False,
                   num_devices=NCORES)
    v_d = nc.declare_dram_parameter("v", [cfg.N, C], F32, isOutput=False)
    meta_d = nc.declare_dram_parameter("meta", [SLOTS, 3 * T], F32, isOutput=False)
    iota_d = nc.declare_dram_parameter("iota", [128, 128], F32, isOutput=False)
    if backend == "dma_gather":
        ix_d = nc.declare_dram_parameter("ix", [SLOTS, T * 8], mybir.dt.int16, isOutput=False)
    y_d = nc.declare_dram_parameter("y", [SLOTS, C], F32, isOutput=True)

    with TileContext(nc) as tc:
        with (
            tc.tile_pool(name="const", bufs=1) as cpool,
            tc.tile_pool(name="sbuf", bufs=2) as pool,
            tc.tile_pool(name="gp", bufs=2) as gpool,
            tc.tile_pool(name="pp", bufs=2) as ppool,
            tc.tile_pool(name="psum", bufs=2, space="PSUM") as psum,
        ):
            iota = cpool.tile([128, 128], F32)
            nc.sync.dma_start(out=iota[:], in_=iota_d[:])

            def body(i):
                m = pool.tile([128, 3 * T], F32, tag="meta")
                nc.sync.dma_start(out=m[:], in_=meta_d[bass.ds(i * 128, 128), :])
                ix = None
                if backend == "dma_gather":
                    ix = pool.tile([128, T * 8], mybir.dt.int16, tag="ix")
                    nc.sync.dma_start(out=ix[:], in_=ix_d[bass.ds(i * 128, 128), :])
                gs = _emit_gathers(nc, cfg, backend, gather_cfg, gpool, v_d, m, i, T, C, ix)
                y_ps = psum.tile([128, C], F32, tag="yps")
                for t in range(T):
                    P_t = ppool.tile([128, 128], F32, tag=f"P{t}")
                    _p_build(nc, P_t, iota, m, T, t)
                    nc.tensor.matmul(out=y_ps[:], lhsT=P_t[:], rhs=gs[t],
                                     start=(t == 0), stop=(t == T - 1))
                y_sb = pool.tile([128, C], F32, tag="ysb")
                nc.vector.tensor_copy(y_sb[:], y_ps[:])
                nc.sync.dma_start(out=y_d[bass.ds(i * 128, 128), :], in_=y_sb[:])

            tc.For_i_unrolled(0, NB, 1, body, max_unroll=unroll)
    nc.finalize()
    return nc


def build_combine(cfg, Cin, Cout, T, relu, unroll=2, backend="indirect", gather_cfg=None):
    """Program B: h = relu(tx0@W0 + tx1@W1 + (2*prop(t1) - tx0)@W2 + b)."""
    NB, SLOTS = cfg.NB, cfg.SLOTS
    nc = bacc.Bacc("TRN2", target_bir_lowering=False, debug=False,
                   num_devices=NCORES)
    v_d = nc.declare_dram_parameter("v", [cfg.N, Cin], F32, isOutput=False)  # t1_full
    x0T_d = nc.declare_dram_parameter("x0T", [Cin, SLOTS], F32, isOutput=False)
    t1T_d = nc.declare_dram_parameter("t1T", [Cin, SLOTS], F32, isOutput=False)
    meta_d = nc.declare_dram_parameter("meta", [SLOTS, 3 * T], F32, isOutput=False)
    iota_d = nc.declare_dram_parameter("iota", [128, 128], F32, isOutput=False)
    wk_d = nc.declare_dram_parameter("wk", [3 * Cin, Cout], F32, isOutput=False)
    bias_d = nc.declare_dram_parameter("bias", [128, Cout], F32, isOutput=False)
    if backend == "dma_gather":
        ix_d = nc.declare_dram_parameter("ix", [SLOTS, T * 8], mybir.dt.int16, isOutput=False)
    h_d = nc.declare_dram_parameter("h", [SLOTS, Cout], F32, isOutput=True)

    with TileContext(nc) as tc:
        with (
            tc.tile_pool(name="const", bufs=1) as cpool,
            tc.tile_pool(name="sbuf", bufs=2) as pool,
            tc.tile_pool(name="gp", bufs=2) as gpool,
            tc.tile_pool(name="pp", bufs=2) as ppool,
            tc.tile_pool(name="psum", bufs=2, space="PSUM") as psum,
        ):
            iota = cpool.tile([128, 128], F32)
            nc.sync.dma_start(out=iota[:], in_=iota_d[:])
            wks = []
            for k in range(3):
                wt = cpool.tile([Cin, Cout], F32, tag=f"w{k}")
                nc.sync.dma_start(out=wt[:], in_=wk_d[k * Cin:(k + 1) * Cin, :])
                wks.append(wt)
            bias = cpool.tile([128, Cout], F32)
            nc.sync.dma_start(out=bias[:], in_=bias_d[:])

            def body(i):
                m = pool.tile([128, 3 * T], F32, tag="meta")
                nc.sync.dma_start(out=m[:], in_=meta_d[bass.ds(i * 128, 128), :])
                ix = None
                if backend == "dma_gather":
                    ix = pool.tile([128, T * 8], mybir.dt.int16, tag="ix")
                    nc.sync.dma_start(out=ix[:], in_=ix_d[bass.ds(i * 128, 128), :])
                gs = _emit_gathers(nc, cfg, backend, gather_cfg, gpool, v_d, m, i, T, Cin, ix)
                s_ps = psum.tile([Cin, 128], F32, tag="sps")
                for t in range(T):
                    P_t = ppool.tile([128, 128], F32, tag=f"P{t}")
                    _p_build(nc, P_t, iota, m, T, t)
                    nc.tensor.matmul(out=s_ps[:], lhsT=gs[t], rhs=P_t[:],
                                     start=(t == 0), stop=(t == T - 1))
                x0T = pool.tile([Cin, 128], F32, tag="x0T")
                nc.sync.dma_start(out=x0T[:], in_=x0T_d[:, bass.ds(i * 128, 128)])
                t1T = pool.tile([Cin, 128], F32, tag="t1T")
                nc.sync.dma_start(out=t1T[:], in_=t1T_d[:, bass.ds(i * 128, 128)])
                # tx2T = 2*s_ps - x0T
                tx2T = pool.tile([Cin, 128], F32, tag="tx2T")
                nc.vector.scalar_tensor_tensor(
                    out=tx2T[:], in0=s_ps[:], scalar=2.0, in1=x0T[:],
                    op0=mybir.AluOpType.mult, op1=mybir.AluOpType.subtract)
                o_ps = psum.tile([128, Cout], F32, tag="ops")
                nc.tensor.matmul(out=o_ps[:], lhsT=x0T[:], rhs=wks[0][:],
                                 start=True, stop=False)
                nc.tensor.matmul(out=o_ps[:], lhsT=t1T[:], rhs=wks[1][:],
                                 start=False, stop=False)
                nc.tensor.matmul(out=o_ps[:], lhsT=tx2T[:], rhs=wks[2][:],
                                 start=False, stop=True)
                h_sb = pool.tile([128, Cout], F32, tag="hsb")
                nc.vector.tensor_tensor(out=h_sb[:], in0=o_ps[:], in1=bias[:],
                                        op=mybir.AluOpType.add)
                if relu:
                    nc.vector.tensor_scalar_max(out=h_sb[:], in0=h_sb[:], scalar1=0.0)
                nc.sync.dma_start(out=h_d[bass.ds(i * 128, 128), :], in_=h_sb[:])

            tc.For_i_unrolled(0, NB, 1, body, max_unroll=unroll)
    nc.finalize()
    return nc


# ------------------------------------------------------ fused single launch

def build_fused(cfg, T, dims=((128, 64, True), (64, 128, True), (128, 256, False)),
                unroll=2, backend="indirect", gather_cfg=None):
    """One program: all 3 layers with on-device AllGather between phases.

    Gather-source id space is the per-core slot layout [NCORES*SLOTS, C]
    (host remaps src ids via host_prep(remap_slots=True)).

    Inputs per core: xg (replicated gather-layout input), x0T (this core's
    input feature-major), meta, iota, [ix], wk{1..3}, bias{1..3}.
    Output: h3 [SLOTS, Cout_last].
    """
    NB, SLOTS = cfg.NB, cfg.SLOTS
    NG = NCORES * SLOTS
    C0 = dims[0][0]
    nc = bacc.Bacc("TRN2", target_bir_lowering=False, debug=False,
                   num_devices=NCORES)
    xg_d = nc.declare_dram_parameter("xg", [NG, C0], F32, isOutput=False)
    x0T_d = nc.declare_dram_parameter("x0T", [C0, SLOTS], F32, isOutput=False)
    meta_d = nc.declare_dram_parameter("meta", [SLOTS, 3 * T], F32, isOutput=False)
    iota_d = nc.declare_dram_parameter("iota", [128, 128], F32, isOutput=False)
    if backend == "dma_gather":
        ix_d = nc.declare_dram_parameter("ix", [SLOTS, T * 8], mybir.dt.int16, isOutput=False)
    wk_ds, bias_ds = [], []
    for li, (Cin, Cout, relu) in enumerate(dims):
        wk_ds.append(nc.declare_dram_parameter(f"wk{li}", [3 * Cin, Cout], F32, isOutput=False))
        bias_ds.append(nc.declare_dram_parameter(f"bias{li}", [128, Cout], F32, isOutput=False))
    out_d = nc.declare_dram_parameter("h3", [SLOTS, dims[-1][1]], F32, isOutput=True)

    # internal DRAM
    t1_s, t1_g, h_s, hT_s, h_g = [], [], [], [], []
    for li, (Cin, Cout, relu) in enumerate(dims):
        t1_s.append(nc.dram_tensor(f"t1s{li}", [SLOTS, Cin], F32))
        t1_g.append(nc.dram_tensor(f"t1g{li}", [NG, Cin], F32, addr_space="Shared"))
        if li < len(dims) - 1:
            h_s.append(nc.dram_tensor(f"hs{li}", [SLOTS, Cout], F32))
            hT_s.append(nc.dram_tensor(f"hTs{li}", [Cout, SLOTS], F32))
            h_g.append(nc.dram_tensor(f"hg{li}", [NG, Cout], F32, addr_space="Shared"))
        else:
            h_s.append(None); hT_s.append(None); h_g.append(None)

    groups = [list(range(NCORES))]

    with TileContext(nc) as tc:
        with (
            tc.tile_pool(name="const", bufs=1) as cpool,
            tc.tile_pool(name="sbuf", bufs=2) as pool,
            tc.tile_pool(name="gp", bufs=2) as gpool,
            tc.tile_pool(name="pp", bufs=2) as ppool,
            tc.tile_pool(name="psum", bufs=2, space="PSUM") as psum,
            tc.tile_pool(name="psumt", bufs=2, space="PSUM") as psumt,
        ):
            iota = cpool.tile([128, 128], F32)
            nc.sync.dma_start(out=iota[:], in_=iota_d[:])
            ident = cpool.tile([128, 128], F32)
            from concourse.masks import make_identity
            make_identity(nc, ident[:])
            wks, biases = [], []
            for li, (Cin, Cout, relu) in enumerate(dims):
                row = []
                for k in range(3):
                    wt = cpool.tile([Cin, Cout], F32, tag=f"w{li}_{k}")
                    nc.sync.dma_start(out=wt[:], in_=wk_ds[li][k * Cin:(k + 1) * Cin, :])
                    row.append(wt)
                wks.append(row)
                bt = cpool.tile([128, Cout], F32, tag=f"b{li}")
                nc.sync.dma_start(out=bt[:], in_=bias_ds[li][:])
                biases.append(bt)

            for li, (Cin, Cout, relu) in enumerate(dims):
                v_prop = xg_d if li == 0 else h_g[li - 1]

                def prop_body(i, li=li, Cin=Cin, v_prop=v_prop):
                    m = pool.tile([128, 3 * T], F32, tag="meta")
                    nc.sync.dma_start(out=m[:], in_=meta_d[bass.ds(i * 128, 128), :])
                    ix = None
                    if backend == "dma_gather":
                        ix = pool.tile([128, T * 8], mybir.dt.int16, tag="ix")
                        nc.sync.dma_start(out=ix[:], in_=ix_d[bass.ds(i * 128, 128), :])
                    gs = _emit_gathers(nc, cfg, backend, gather_cfg, gpool, v_prop, m, i, T, Cin, ix)
                    y_ps = psum.tile([128, Cin], F32, tag="yps")
                    for t in range(T):
                        P_t = ppool.tile([128, 128], F32, tag=f"P{t}")
                        _p_build(nc, P_t, iota, m, T, t)
                        nc.tensor.matmul(out=y_ps[:], lhsT=P_t[:], rhs=gs[t],
                                         start=(t == 0), stop=(t == T - 1))
                    y_sb = pool.tile([128, Cin], F32, tag="ysb")
                    nc.vector.tensor_copy(y_sb[:], y_ps[:])
                    nc.sync.dma_start(out=t1_s[li][bass.ds(i * 128, 128), :], in_=y_sb[:])

                tc.For_i_unrolled(0, NB, 1, prop_body, max_unroll=unroll)

                nc.gpsimd.collective_compute(
                    "AllGather", mybir.AluOpType.bypass, replica_groups=groups,
                    ins=[t1_s[li][:]], outs=[t1_g[li][:]])

                def comb_body(i, li=li, Cin=Cin, Cout=Cout, relu=relu):
                    m = pool.tile([128, 3 * T], F32, tag="meta")
                    nc.sync.dma_start(out=m[:], in_=meta_d[bass.ds(i * 128, 128), :])
                    ix = None
                    if backend == "dma_gather":
                        ix = pool.tile([128, T * 8], mybir.dt.int16, tag="ix")
                        nc.sync.dma_start(out=ix[:], in_=ix_d[bass.ds(i * 128, 128), :])
                    gs = _emit_gathers(nc, cfg, backend, gather_cfg, gpool, t1_g[li], m, i, T, Cin, ix)
                    s_ps = psum.tile([Cin, 128], F32, tag="sps")
                    for t in range(T):
                        P_t = ppool.tile([128, 128], F32, tag=f"P{t}")
                        _p_build(nc, P_t, iota, m, T, t)
                        nc.tensor.matmul(out=s_ps[:], lhsT=gs[t], rhs=P_t[:],
                                         start=(t == 0), stop=(t == T - 1))
                    # x0T: layer 0 from input; else transpose of h_s[li-1] block
                    x0T = pool.tile([Cin, 128], F32, tag="x0T")
                    if li == 0:
                        nc.sync.dma_start(out=x0T[:], in_=x0T_d[:, bass.ds(i * 128, 128)])
                    else:
                        xb = pool.tile([128, Cin], F32, tag="xb")
                        nc.sync.dma_start(out=xb[:], in_=h_s[li - 1][bass.ds(i * 128, 128), :])
                        xT_ps = psumt.tile([Cin, 128], F32, tag="xTps")
                        nc.tensor.transpose(out=xT_ps[:], in_=xb[:], identity=ident[:])
                        nc.vector.tensor_copy(x0T[:], xT_ps[:])
                    # t1T: transpose of t1_s block
                    t1b = pool.tile([128, Cin], F32, tag="t1b")
                    nc.sync.dma_start(out=t1b[:], in_=t1_s[li][bass.ds(i * 128, 128), :])
                    t1T_ps = psumt.tile([Cin, 128], F32, tag="t1Tps")
                    nc.tensor.transpose(out=t1T_ps[:], in_=t1b[:], identity=ident[:])
                    t1T = pool.tile([Cin, 128], F32, tag="t1T")
                    nc.vector.tensor_copy(t1T[:], t1T_ps[:])
                    tx2T = pool.tile([Cin, 128], F32, tag="tx2T")
                    nc.vector.scalar_tensor_tensor(
                        out=tx2T[:], in0=s_ps[:], scalar=2.0, in1=x0T[:],
                        op0=mybir.AluOpType.mult, op1=mybir.AluOpType.subtract)
                    o_ps = psum.tile([128, Cout], F32, tag="ops")
                    nc.tensor.matmul(out=o_ps[:], lhsT=x0T[:], rhs=wks[li][0][:],
                                     start=True, stop=False)
                    nc.tensor.matmul(out=o_ps[:], lhsT=t1T[:], rhs=wks[li][1][:],
                                     start=False, stop=False)
                    nc.tensor.matmul(out=o_ps[:], lhsT=tx2T[:], rhs=wks[li][2][:],
                                     start=False, stop=True)
                    h_sb = pool.tile([128, Cout], F32, tag="hsb")
                    nc.vector.tensor_tensor(out=h_sb[:], in0=o_ps[:], in1=biases[li][:],
                                            op=mybir.AluOpType.add)
                    if relu:
                        nc.vector.tensor_scalar_max(out=h_sb[:], in0=h_sb[:], scalar1=0.0)
                    if li == len(dims) - 1:
                        nc.sync.dma_start(out=out_d[bass.ds(i * 128, 128), :], in_=h_sb[:])
                    else:
                        nc.sync.dma_start(out=h_s[li][bass.ds(i * 128, 128), :], in_=h_sb[:])

                tc.For_i_unrolled(0, NB, 1, comb_body, max_unroll=unroll)

                if li < len(dims) - 1:
                    nc.gpsimd.collective_compute(
                        "AllGather", mybir.AluOpType.bypass, replica_groups=groups,
                        ins=[h_s[li][:]], outs=[h_g[li][:]])
    nc.finalize()
    return nc


# ------------------------------------------------------------- full model

class GnnModel:
    """Builds/caches the 5 programs + runners; executes the 3-layer model."""

    def __init__(self, cfg, T_fix, dims=((128, 64, True), (64, 128, True), (128, 256, False)),
                 unroll=2, make_runner=True, backend="indirect", gather_cfg=None):
        self.cfg = cfg
        self.T = T_fix
        self.dims = dims
        self.backend = backend
        self.gather_cfg = gather_cfg
        self.progs = {}
        key_done = set()
        for (Cin, Cout, relu) in dims:
            if ("A", Cin) not in key_done:
                nc = build_prop(cfg, Cin, T_fix, unroll, backend, gather_cfg)
                self.progs[("A", Cin)] = Runner(nc) if make_runner else nc
                key_done.add(("A", Cin))
            k = ("B", Cin, Cout, relu)
            nc = build_combine(cfg, Cin, Cout, T_fix, relu, unroll, backend, gather_cfg)
            self.progs[k] = Runner(nc) if make_runner else nc
            key_done.add(k)

    def run(self, x, metas, weights, timing=None, idx16s=None):
        """x: [N, C0] f32; metas: per-core packed meta; weights: [(W,b)]*3."""
        import time
        cfg = self.cfg
        iota = iota_host()
        extra = [{} for _ in range(NCORES)]
        if self.backend == "dma_gather":
            extra = [{"ix": idx16s[c]} for c in range(NCORES)]
        h_full = np.asarray(x, np.float32)          # replicated input, node-major
        h_slots = [None] * NCORES                   # per-core slot-order tx0
        for c in range(NCORES):
            sl = np.zeros((cfg.SLOTS, h_full.shape[1]), np.float32)
            sl[:cfg.NPC] = h_full[c * cfg.NPC:(c + 1) * cfg.NPC]
            h_slots[c] = sl

        out_slots = None
        self.call_times = []
        for li, (Cin, Cout, relu) in enumerate(self.dims):
            (W, b) = weights[li]
            t0 = time.perf_counter()
            # ---- program A: tx1 = prop(h)
            rA = self.progs[("A", Cin)]
            inA = [{"v": h_full, "meta": metas[c], "iota": iota, **extra[c]} for c in range(NCORES)]
            devA = rA.put_inputs(inA)
            tc0 = time.perf_counter()
            outsA = rA(devA)
            self.call_times.append(time.perf_counter() - tc0)
            resA = rA.results(outsA)
            y_slots = [resA[c]["y"] for c in range(NCORES)]
            t1_full = np.concatenate([y[:cfg.NPC] for y in y_slots], axis=0)
            # ---- program B: combine
            rB = self.progs[("B", Cin, Cout, relu)]
            bias_rep = np.broadcast_to(b.astype(np.float32), (128, Cout)).copy()
            wk = np.ascontiguousarray(W.astype(np.float32).reshape(3 * Cin, Cout))
            inB = []
            for c in range(NCORES):
                inB.append({
                    "v": t1_full,
                    "x0T": np.ascontiguousarray(h_slots[c].T),
                    "t1T": np.ascontiguousarray(y_slots[c].T),
                    "meta": metas[c], "iota": iota,
                    "wk": wk, "bias": bias_rep, **extra[c],
                })
            devB = rB.put_inputs(inB)
            tc0 = time.perf_counter()
            outsB = rB(devB)
            self.call_times.append(time.perf_counter() - tc0)
            resB = rB.results(outsB)
            out_slots = [resB[c]["h"] for c in range(NCORES)]
            if timing is not None:
                timing.append(time.perf_counter() - t0)
            if li < len(self.dims) - 1:
                h_full = np.concatenate([o[:cfg.NPC] for o in out_slots], axis=0)
                h_slots = out_slots

        return np.concatenate([o[:cfg.NPC] for o in out_slots], axis=0)


# ------------------------------------------------------------- numpy oracle

def numpy_reference(x, edge_index, weights):
    N = x.shape[0]
    src = np.asarray(edge_index[0], np.int64)
    dst = np.asarray(edge_index[1], np.int64)
    mask = src != dst
    deg = np.bincount(src[mask], minlength=N).astype(np.float32)
    dinv = np.where(deg > 0, (1.0 / np.sqrt(np.maximum(deg, 1.0))).astype(np.float32), 0.0).astype(np.float32)
    w = (-dinv[src] * dinv[dst] * mask).astype(np.float32)

    def prop(h):
        out = np.zeros_like(h)
        np.add.at(out, dst, w[:, None] * h[src])
        return out

    h = x.astype(np.float32)
    for li, (W, b) in enumerate(weights):
        tx0 = h
        tx1 = prop(h)
        tx2 = 2.0 * prop(tx1) - tx0
        h = tx0 @ W[0] + tx1 @ W[1] + tx2 @ W[2] + b
        if li < len(weights) - 1:
            h = np.maximum(h, 0.0)
    return h


# ------------------------------------------------------------------ entry

N_NODES = 100000
BACKEND = "indirect"
_cache = {}
_LAST = None


def kernel(x, edge_index, batch, W1, b1, W2, b2, W3, b3):
    global _LAST
    cfg = Cfg(n_nodes=N_NODES, npc=N_NODES // NCORES)
    weights = [
        (np.asarray(W1, np.float32), np.asarray(b1, np.float32)),
        (np.asarray(W2, np.float32), np.asarray(b2, np.float32)),
        (np.asarray(W3, np.float32), np.asarray(b3, np.float32)),
    ]
    metas, T = host_prep(cfg, edge_index)
    key = (BACKEND, T)
    if key not in _cache:
        _cache[key] = GnnModel(cfg, T)
    model = _cache[key]
    out = model.run(np.asarray(x, np.float32), metas, weights)
    _LAST = (model, metas, weights, None)
    return out.astype(np.float32)



# revision 2
# speedup vs baseline: 148.7471x; 148.7471x over previous
"""ChebConv(K=3) x3 GNN encoder on 8 trn2 NeuronCores (Bass/Tile).

Fused single-launch version: one Bass program runs all 3 layers with
on-device AllGather between phases.  Host stages per-core x shards +
edge metadata once; warm runs re-stage only x (51MB) and fetch the
output shards.

Strategy per core: 98 blocks x 128 dst slots; per 128-edge tile an
indirect-DMA row gather plus a one-hot selection matrix (iota==dloc)*w
built on DVE, reduced on the PE via P^T @ M with PSUM accumulation;
dense matmuls for the Chebyshev combine.
"""
import numpy as np
import jax
from jax.sharding import Mesh, PartitionSpec, NamedSharding
from jax.experimental.shard_map import shard_map

import concourse.bass as bass
import concourse.bacc as bacc
import concourse.mybir as mybir
from concourse.tile import TileContext
from concourse import bass2jax
from concourse.masks import make_identity

F32 = mybir.dt.float32
I32 = mybir.dt.int32
NCORES = 8


class Runner:
    def __init__(self, nc, n_cores=8):
        bass2jax.install_neuronx_cc_hook()
        self.nc = nc
        self.n_cores = n_cores
        partition_name = (
            nc.partition_id_tensor.name if nc.partition_id_tensor else None
        )
        in_names, out_names, out_avals = [], [], []
        for alloc in nc.m.functions[0].allocations:
            if not isinstance(alloc, mybir.MemoryLocationSet):
                continue
            name = alloc.memorylocations[0].name
            if alloc.kind == "ExternalInput":
                if name != partition_name:
                    in_names.append(name)
            elif alloc.kind == "ExternalOutput":
                out_names.append(name)
                out_avals.append(
                    jax.core.ShapedArray(
                        tuple(alloc.tensor_shape), mybir.dt.np(alloc.dtype)
                    )
                )
        self.in_names, self.out_names, self.out_avals = in_names, out_names, out_avals
        n_params = len(in_names)
        all_in_names = in_names + out_names + (
            [partition_name] if partition_name else []
        )

        def _body(*args):
            operands = list(args)
            if partition_name is not None:
                operands.append(bass2jax.partition_id_tensor())
            outs = bass2jax._bass_exec_p.bind(
                *operands,
                out_avals=tuple(out_avals),
                in_names=tuple(all_in_names),
                out_names=tuple(out_names),
                lowering_input_output_aliases=(),
                sim_require_finite=True,
                sim_require_nnan=True,
                nc=nc,
            )
            return tuple(outs)

        devices = jax.devices()[:n_cores]
        self.mesh = Mesh(np.asarray(devices), ("core",))
        self.sharding = NamedSharding(self.mesh, PartitionSpec("core"))
        nin = n_params + len(out_names)
        self.fn = jax.jit(
            shard_map(
                _body,
                mesh=self.mesh,
                in_specs=(PartitionSpec("core"),) * nin,
                out_specs=(PartitionSpec("core"),) * len(out_names),
                check_rep=False,
            ),
            keep_unused=True,
        )

    def put_one(self, name, arrs):
        """arrs: list of per-core arrays (len n_cores) or one replicated."""
        if not isinstance(arrs, (list, tuple)):
            arrs = [arrs] * self.n_cores
        cat = np.concatenate([np.asarray(a) for a in arrs], axis=0)
        return jax.device_put(cat, self.sharding)

    def zeros(self):
        return [
            jax.device_put(
                np.zeros((self.n_cores * a.shape[0], *a.shape[1:]), a.dtype),
                self.sharding,
            )
            for a in self.out_avals
        ]


class Cfg:
    def __init__(self, n_nodes, npc, blk=128):
        assert npc * NCORES == n_nodes
        self.N = n_nodes
        self.NPC = npc
        self.BLK = blk
        self.NB = -(-npc // blk)          # blocks per core
        self.SLOTS = self.NB * blk        # slots per core (>= npc)


# ---------------------------------------------------------------- host prep

def host_prep(cfg, edge_index):
    """Bin edges by (core, block) of dst, pad to T_fix 128-edge tiles.

    Gather-source ids are remapped to the per-core slot layout
    [NCORES*SLOTS, C].  Returns (metas, T_fix); metas[c] is
    [NB*128, 3*T_fix] f32; cols [0:T) src ids (int32 bitcast),
    [T:2T) dloc f32, [2T:3T) w f32.
    """
    N, NPC, BLK, NB = cfg.N, cfg.NPC, cfg.BLK, cfg.NB
    src = np.asarray(edge_index[0], dtype=np.int64)
    dst = np.asarray(edge_index[1], dtype=np.int64)
    mask = src != dst
    deg = np.bincount(src[mask], minlength=N).astype(np.float32)
    dinv = np.where(deg > 0, (1.0 / np.sqrt(np.maximum(deg, 1.0))).astype(np.float32), 0.0).astype(np.float32)
    w_all = (-dinv[src] * dinv[dst]).astype(np.float32)

    src = src[mask]
    dst = dst[mask]
    w = w_all[mask]

    order = np.argsort(dst, kind="stable")
    src, dst, w = src[order], dst[order], w[order]

    # slot-layout remap of gather sources
    src = (src // NPC) * cfg.SLOTS + (src % NPC)

    core = dst // NPC
    core_starts = np.searchsorted(core, np.arange(NCORES + 1))

    # per (core, block) counts to get global T_fix
    gb = (dst // BLK) if NPC % BLK == 0 else (core * NB + (dst - core * NPC) // BLK)
    cnt = np.bincount(gb, minlength=NCORES * NB)
    T_fix = int(-(-cnt.max() // 128))

    metas = []
    for c in range(NCORES):
        s, e = core_starts[c], core_starts[c + 1]
        cs, cd, cw = src[s:e], dst[s:e], w[s:e]
        b = (cd - c * NPC) // BLK
        dloc = (cd - c * NPC) % BLK
        bstart = np.searchsorted(b, np.arange(NB + 1))
        meta = np.zeros((NB, 128, 3 * T_fix), np.float32)
        idx_i32 = np.zeros((NB, 128, T_fix), np.int32)
        # vectorized scatter over the whole core
        n_in_blk = np.diff(bstart)
        pos = np.arange(len(cs)) - np.repeat(bstart[:-1], n_in_blk)
        t_i = pos // 128
        p_i = pos % 128
        bi = np.repeat(np.arange(NB), n_in_blk)
        idx_i32[bi, p_i, t_i] = cs.astype(np.int32)
        meta[bi, p_i, T_fix + t_i] = dloc.astype(np.float32)
        meta[bi, p_i, 2 * T_fix + t_i] = cw
        meta[:, :, 0:T_fix] = idx_i32.view(np.float32)
        metas.append(meta.reshape(NB * 128, 3 * T_fix))
    return metas, T_fix


def iota_host():
    return np.broadcast_to(np.arange(128, dtype=np.float32), (128, 128)).copy()


# ------------------------------------------------------------- program

def _p_build(nc, P_t, iota, m, T, t):
    """P_t[p, c] = (iota[c] == dloc[p]) * w[p]"""
    nc.vector.tensor_scalar(
        out=P_t[:], in0=iota[:],
        scalar1=m[:, T + t:T + t + 1],
        scalar2=m[:, 2 * T + t:2 * T + t + 1],
        op0=mybir.AluOpType.is_equal,
        op1=mybir.AluOpType.mult,
    )


def build_fused(cfg, T, dims=((128, 64, True), (64, 128, True), (128, 256, False)),
                unroll=2, no_gather=False, no_coll=False, reps=1):
    """One program: all 3 layers with on-device AllGather between phases.

    Inputs per core: xs (own x shard in slot layout [SLOTS, C0]), meta,
    iota, wk{0..2} [3*Cin, Cout], bias{0..2} [128, Cout].
    Output: h3 [SLOTS, Cout_last].

    no_gather/no_coll: timing-ablation variants (numerically wrong).
    reps: emit the whole model body N times (for dispatch-free timing).
    """
    NB, SLOTS = cfg.NB, cfg.SLOTS
    NG = NCORES * SLOTS
    C0 = dims[0][0]
    nc = bacc.Bacc("TRN2", target_bir_lowering=False, debug=False,
                   num_devices=NCORES)

    def gather_tile(gpool, v_ap, m, t, Cin):
        g = gpool.tile([128, Cin], F32, tag=f"g{t}")
        if no_gather:
            nc.sync.dma_start(out=g[:], in_=v_ap[t * 128:(t + 1) * 128, :])
        else:
            nc.gpsimd.indirect_dma_start(
                out=g[:], out_offset=None, in_=v_ap[:],
                in_offset=bass.IndirectOffsetOnAxis(
                    ap=m[:, t:t + 1].bitcast(I32), axis=0),
            )
        return g

    def allgather(src, dst):
        if no_coll:
            nc.sync.dma_start(out=dst[0:src.shape[0], :], in_=src[:])
        else:
            nc.gpsimd.collective_compute(
                "AllGather", mybir.AluOpType.bypass,
                replica_groups=[list(range(NCORES))],
                ins=[src[:]], outs=[dst[:]])

    xs_d = nc.declare_dram_parameter("xs", [SLOTS, C0], F32, isOutput=False)
    meta_d = nc.declare_dram_parameter("meta", [SLOTS, 3 * T], F32, isOutput=False)
    iota_d = nc.declare_dram_parameter("iota", [128, 128], F32, isOutput=False)
    wk_ds, bias_ds = [], []
    for li, (Cin, Cout, relu) in enumerate(dims):
        wk_ds.append(nc.declare_dram_parameter(f"wk{li}", [3 * Cin, Cout], F32, isOutput=False))
        bias_ds.append(nc.declare_dram_parameter(f"bias{li}", [128, Cout], F32, isOutput=False))
    out_d = nc.declare_dram_parameter("h3", [SLOTS, dims[-1][1]], F32, isOutput=True)

    # internal DRAM
    xb = nc.dram_tensor("xb", [SLOTS, C0], F32)
    xg = nc.dram_tensor("xg", [NG, C0], F32, addr_space="Shared")
    t1_s, t1_g, h_s, h_g = [], [], [], []
    for li, (Cin, Cout, relu) in enumerate(dims):
        t1_s.append(nc.dram_tensor(f"t1s{li}", [SLOTS, Cin], F32))
        t1_g.append(nc.dram_tensor(f"t1g{li}", [NG, Cin], F32, addr_space="Shared"))
        if li < len(dims) - 1:
            h_s.append(nc.dram_tensor(f"hs{li}", [SLOTS, Cout], F32))
            h_g.append(nc.dram_tensor(f"hg{li}", [NG, Cout], F32, addr_space="Shared"))
        else:
            h_s.append(None); h_g.append(None)

    with TileContext(nc) as tc:
        with tc.tile_pool(name="const", bufs=1) as cpool:
            iota = cpool.tile([128, 128], F32)
            nc.sync.dma_start(out=iota[:], in_=iota_d[:])
            ident = cpool.tile([128, 128], F32)
            make_identity(nc, ident[:])
            wks, biases = [], []
            for li, (Cin, Cout, relu) in enumerate(dims):
                row = []
                for k in range(3):
                    wt = cpool.tile([Cin, Cout], F32, tag=f"w{li}_{k}")
                    nc.sync.dma_start(out=wt[:], in_=wk_ds[li][k * Cin:(k + 1) * Cin, :])
                    row.append(wt)
                wks.append(row)
                bt = cpool.tile([128, Cout], F32, tag=f"b{li}")
                nc.sync.dma_start(out=bt[:], in_=bias_ds[li][:])
                biases.append(bt)

            def emit_prop(rep, li, Cin, v_prop):
                with (
                    tc.tile_pool(name=f"pl{rep}_{li}", bufs=2) as pool,
                    tc.tile_pool(name=f"gl{rep}_{li}", bufs=2) as gpool,
                    tc.tile_pool(name=f"ppl{rep}_{li}", bufs=2) as ppool,
                    tc.tile_pool(name=f"psl{rep}_{li}", bufs=2, space="PSUM") as psum,
                ):
                    def prop_body(i):
                        m = pool.tile([128, 3 * T], F32, tag="meta")
                        nc.sync.dma_start(out=m[:], in_=meta_d[bass.ds(i * 128, 128), :])
                        gs = [gather_tile(gpool, v_prop, m, t, Cin) for t in range(T)]
                        y_ps = psum.tile([128, Cin], F32, tag="yps")
                        for t in range(T):
                            P_t = ppool.tile([128, 128], F32, tag=f"P{t}")
                            _p_build(nc, P_t, iota, m, T, t)
                            nc.tensor.matmul(out=y_ps[:], lhsT=P_t[:], rhs=gs[t][:],
                                             start=(t == 0), stop=(t == T - 1))
                        y_sb = pool.tile([128, Cin], F32, tag="ysb")
                        nc.vector.tensor_copy(y_sb[:], y_ps[:])
                        nc.sync.dma_start(out=t1_s[li][bass.ds(i * 128, 128), :], in_=y_sb[:])

                    tc.For_i_unrolled(0, NB, 1, prop_body, max_unroll=unroll)

            def emit_combine(rep, li, Cin, Cout, relu, x0_src):
                with (
                    tc.tile_pool(name=f"cl{rep}_{li}", bufs=2) as pool,
                    tc.tile_pool(name=f"cgl{rep}_{li}", bufs=2) as gpool,
                    tc.tile_pool(name=f"cpl{rep}_{li}", bufs=2) as ppool,
                    tc.tile_pool(name=f"cs{rep}_{li}", bufs=2, space="PSUM") as psum,
                    tc.tile_pool(name=f"ct{rep}_{li}", bufs=2, space="PSUM") as psumt,
                ):
                    def comb_body(i):
                        m = pool.tile([128, 3 * T], F32, tag="meta")
                        nc.sync.dma_start(out=m[:], in_=meta_d[bass.ds(i * 128, 128), :])
                        gs = [gather_tile(gpool, t1_g[li], m, t, Cin) for t in range(T)]
                        s_ps = psum.tile([Cin, 128], F32, tag="sps")
                        for t in range(T):
                            P_t = ppool.tile([128, 128], F32, tag=f"P{t}")
                            _p_build(nc, P_t, iota, m, T, t)
                            nc.tensor.matmul(out=s_ps[:], lhsT=gs[t][:], rhs=P_t[:],
                                             start=(t == 0), stop=(t == T - 1))
                        # x0T via on-device transpose of the x0 block
                        xb_t = pool.tile([128, Cin], F32, tag="xb")
                        nc.sync.dma_start(out=xb_t[:], in_=x0_src[bass.ds(i * 128, 128), :])
                        xT_ps = psumt.tile([Cin, 128], F32, tag="xTps")
                        nc.tensor.transpose(out=xT_ps[:], in_=xb_t[:], identity=ident[:])
                        x0T = pool.tile([Cin, 128], F32, tag="x0T")
                        nc.vector.tensor_copy(x0T[:], xT_ps[:])
                        # t1T via transpose of t1_s block
                        t1b = pool.tile([128, Cin], F32, tag="t1b")
                        nc.sync.dma_start(out=t1b[:], in_=t1_s[li][bass.ds(i * 128, 128), :])
                        t1T_ps = psumt.tile([Cin, 128], F32, tag="t1Tps")
                        nc.tensor.transpose(out=t1T_ps[:], in_=t1b[:], identity=ident[:])
                        t1T = pool.tile([Cin, 128], F32, tag="t1T")
                        nc.vector.tensor_copy(t1T[:], t1T_ps[:])
                        # tx2T = 2*prop(t1) - x0
                        tx2T = pool.tile([Cin, 128], F32, tag="tx2T")
                        nc.vector.scalar_tensor_tensor(
                            out=tx2T[:], in0=s_ps[:], scalar=2.0, in1=x0T[:],
                            op0=mybir.AluOpType.mult, op1=mybir.AluOpType.subtract)
                        o_ps = psum.tile([128, Cout], F32, tag="ops")
                        nc.tensor.matmul(out=o_ps[:], lhsT=x0T[:], rhs=wks[li][0][:],
                                         start=True, stop=False)
                        nc.tensor.matmul(out=o_ps[:], lhsT=t1T[:], rhs=wks[li][1][:],
                                         start=False, stop=False)
                        nc.tensor.matmul(out=o_ps[:], lhsT=tx2T[:], rhs=wks[li][2][:],
                                         start=False, stop=True)
                        h_sb = pool.tile([128, Cout], F32, tag="hsb")
                        nc.vector.tensor_tensor(out=h_sb[:], in0=o_ps[:], in1=biases[li][:],
                                                op=mybir.AluOpType.add)
                        if relu:
                            nc.vector.tensor_scalar_max(out=h_sb[:], in0=h_sb[:], scalar1=0.0)
                        if li == len(dims) - 1:
                            nc.sync.dma_start(out=out_d[bass.ds(i * 128, 128), :], in_=h_sb[:])
                        else:
                            nc.sync.dma_start(out=h_s[li][bass.ds(i * 128, 128), :], in_=h_sb[:])

                    tc.For_i_unrolled(0, NB, 1, comb_body, max_unroll=unroll)

            for rep in range(reps):
                nc.sync.dma_start(out=xb[:], in_=xs_d[:])
                allgather(xb, xg)
                for li, (Cin, Cout, relu) in enumerate(dims):
                    v_prop = xg if li == 0 else h_g[li - 1]
                    x0_src = xs_d if li == 0 else h_s[li - 1]
                    emit_prop(rep, li, Cin, v_prop)
                    allgather(t1_s[li], t1_g[li])
                    emit_combine(rep, li, Cin, Cout, relu, x0_src)
                    if li < len(dims) - 1:
                        allgather(h_s[li], h_g[li])
    nc.finalize()
    return nc


# ------------------------------------------------------------- full model

class FusedModel:
    """Single fused program; constants staged once, warm runs re-stage x only."""

    def __init__(self, cfg, T, dims=((128, 64, True), (64, 128, True), (128, 256, False)),
                 unroll=2, **build_kw):
        self.cfg = cfg
        self.T = T
        self.dims = dims
        nc = build_fused(cfg, T, dims, unroll, **build_kw)
        self.runner = Runner(nc)
        self.dev_const = None
        self.dev_zero = None

    def stage_const(self, metas, weights):
        r = self.runner
        d = {}
        d["meta"] = r.put_one("meta", metas)
        d["iota"] = r.put_one("iota", iota_host())
        for li, (W, b) in enumerate(weights):
            Cin, Cout = self.dims[li][0], self.dims[li][1]
            wk = np.ascontiguousarray(np.asarray(W, np.float32).reshape(3 * Cin, Cout))
            bias_rep = np.broadcast_to(np.asarray(b, np.float32), (128, Cout)).copy()
            d[f"wk{li}"] = r.put_one(f"wk{li}", wk)
            d[f"bias{li}"] = r.put_one(f"bias{li}", bias_rep)
        self.dev_const = d
        self.dev_zero = r.zeros()

    def stage_x(self, x):
        cfg = self.cfg
        xs = np.zeros((NCORES, cfg.SLOTS, x.shape[1]), np.float32)
        xs[:, :cfg.NPC] = np.asarray(x, np.float32).reshape(NCORES, cfg.NPC, -1)
        return jax.device_put(xs.reshape(NCORES * cfg.SLOTS, -1), self.runner.sharding)

    def run(self, x):
        cfg = self.cfg
        r = self.runner
        dev_x = self.stage_x(x)
        args = [self.dev_const[n] if n != "xs" else dev_x for n in r.in_names]
        outs = r.fn(*args, *self.dev_zero)
        jax.block_until_ready(outs)
        h3 = np.asarray(outs[0]).reshape(NCORES, cfg.SLOTS, -1)
        return np.ascontiguousarray(h3[:, :cfg.NPC]).reshape(cfg.N, -1)


# ------------------------------------------------------------- numpy oracle

def numpy_reference(x, edge_index, weights):
    N = x.shape[0]
    src = np.asarray(edge_index[0], np.int64)
    dst = np.asarray(edge_index[1], np.int64)
    mask = src != dst
    deg = np.bincount(src[mask], minlength=N).astype(np.float32)
    dinv = np.where(deg > 0, (1.0 / np.sqrt(np.maximum(deg, 1.0))).astype(np.float32), 0.0).astype(np.float32)
    w = (-dinv[src] * dinv[dst] * mask).astype(np.float32)

    def prop(h):
        out = np.zeros_like(h)
        np.add.at(out, dst, w[:, None] * h[src])
        return out

    h = x.astype(np.float32)
    for li, (W, b) in enumerate(weights):
        tx0 = h
        tx1 = prop(h)
        tx2 = 2.0 * prop(tx1) - tx0
        h = tx0 @ W[0] + tx1 @ W[1] + tx2 @ W[2] + b
        if li < len(weights) - 1:
            h = np.maximum(h, 0.0)
    return h


# ------------------------------------------------------------------ entry

N_NODES = 100000
_cache = {}
_LAST = None


def kernel(x, edge_index, batch, W1, b1, W2, b2, W3, b3):
    global _LAST
    cfg = Cfg(n_nodes=N_NODES, npc=N_NODES // NCORES)
    weights = [
        (np.asarray(W1, np.float32), np.asarray(b1, np.float32)),
        (np.asarray(W2, np.float32), np.asarray(b2, np.float32)),
        (np.asarray(W3, np.float32), np.asarray(b3, np.float32)),
    ]
    metas, T = host_prep(cfg, edge_index)
    key = ("fused", T)
    if key not in _cache:
        _cache[key] = FusedModel(cfg, T)
    model = _cache[key]
    model.stage_const(metas, weights)
    out = model.run(np.asarray(x, np.float32))
    _LAST = model
    return out.astype(np.float32)
